# revision 5
# baseline (speedup 1.0000x reference)
"""Trainium2 Bass kernel: CNN-feature SoftDTW few-shot classifier.

Computes, for Q=100 query sequences and S=25 support sequences (T=128 steps,
D=2048 features): pairwise squared-euclidean cost matrices, soft-DTW alignment
cost per (query, support) pair, then per-class mean distances -> logits.

Key numerical fact: with gamma=0.1 and cost magnitudes ~4096, the reference's
fp32 softmin is bitwise the hard min (exp((m-x)/gamma) underflows for every
non-minimal branch), so the DP is computed with min/add only. Each DP row is
one `tensor_tensor_scan(op0=min, op1=add)` instruction.

Sharding: data-parallel over queries, 13 per core (Q padded 100->104),
supports replicated. Per core:
  - PE: xy = (-2X)@Y^T in bf16 (16 K-tiles) + fp32r rank-2 update adding
    x2[i] + y2[s,j] exactly -> full cost matrix D in PSUM (fp32).
  - ACT: evacuate PSUM -> SBUF; DMA D to DRAM scratch per query.
  - DMA gather: re-layout D from [i, (s,j)] to [(q,s)-partition, i-window, j].
  - DVE: hard-DTW rows: min(up,diag) + scan, 128 rows x 3 pair-streams.
Host: bf16 packing/transposes, x2/y2 sums, final class-mean logits.
"""

import sys

for _p in ("/opt/trn_rl_repo",):
    if _p not in sys.path:
        sys.path.insert(0, _p)

import numpy as np
import ml_dtypes

# Problem shape (hardcoded: harness runs kernel.py standalone)
Q, S, T, DD = 100, 25, 128, 2048
NCORES = 8
QC = 13                 # queries per core; Q padded to 104
QPAD = QC * NCORES
NK = DD // 128          # 16 bf16 contraction tiles
SJ = S * T              # 3200 = flattened (support, j)
B = QC * S              # 325 pairs per core
# DP pair-tile streams aligned to query boundaries (offset, count):
# a stream's first row can run as soon as its LAST query's cost matrix is
# in DRAM, so query-aligned splits start streams as early as possible.
PT = [(0, 125), (125, 100), (225, 100)]   # q0-4 | q5-8 | q9-12
# 5/4/4 split: stream deps land at ~157/248/338us (query cadence ~23us), so
# the DVE picks up each stream right as its last query's D lands — the final
# stream starts at its data dependency instead of queueing behind stream 1.
W = 16                  # DP row-window per gather DMA
CH = 512                # matmul moving-chunk / PSUM bank width
# all chunks >=256 so the fp32r rank-2 matmul stays at 1 cycle/row
_CW = [512, 512, 512, 512, 512, 384, 256]
CHUNKS = [(sum(_CW[:i]), w) for i, w in enumerate(_CW)]
assert sum(_CW) == SJ
BIG = 1e10

_built = None          # cached compiled Bass program
_last_result = None    # last BassKernelResults (exec_time_ns when traced)
_predicted_ns = None   # Tile cost-model makespan of the per-core program


def _build():
    import concourse.bacc as bacc
    import concourse.mybir as mybir
    import concourse.tile as tile

    f32 = mybir.dt.float32
    f32r = mybir.dt.float32r
    bf16 = mybir.dt.bfloat16
    fp8 = mybir.dt.float8e4
    DR = mybir.MatmulPerfMode.DoubleRow
    MIN = mybir.AluOpType.min
    ADD = mybir.AluOpType.add

    global _predicted_ns
    nc = bacc.Bacc("TRN2", debug=False)

    xt_d = nc.dram_tensor("xt", [QC, 128, NK * T], fp8, kind="ExternalInput")
    yt_d = nc.dram_tensor("yt", [128, NK * SJ], fp8, kind="ExternalInput")
    augl_d = nc.dram_tensor("augl", [QC, 2, T], f32r, kind="ExternalInput")
    augr_d = nc.dram_tensor("augr", [2, SJ], f32r, kind="ExternalInput")
    out_d = nc.dram_tensor("out_cd", [QC, S], f32, kind="ExternalOutput")
    # cost matrices staged pair-major: [q, s, i, j] -> window reads are
    # single 3-dim APs with 8KB-contiguous runs per pair
    dsc = nc.dram_tensor("dsc", [QC, S, T, T], f32)
    dsc_p = dsc[:].rearrange("q s i j -> (q s) i j")

    with tile.TileContext(nc) as tc:
        with (
            tc.tile_pool(name="const", bufs=1) as constp,
            tc.tile_pool(name="xq", bufs=2) as xqp,
            tc.tile_pool(name="augq", bufs=2) as augqp,
            tc.tile_pool(name="psum", bufs=8, space="PSUM") as psump,
            tc.tile_pool(name="dq", bufs=1) as dqp,
            tc.tile_pool(name="ga", bufs=2) as gap,      # pair-tiles 0 and 2
            tc.tile_pool(name="gb", bufs=2) as gbp,      # pair-tile 1
            tc.tile_pool(name="muda", bufs=2) as mudap,
            tc.tile_pool(name="mudb", bufs=2) as mudbp,
            tc.tile_pool(name="dp", bufs=1) as dpp,
        ):
            # q0's operands first on the ACT queue (ahead of the yt halves).
            xt0_sb = xqp.tile([128, NK * T], fp8, tag="xt")
            nc.scalar.dma_start(xt0_sb[:], xt_d[0])
            augl0_sb = augqp.tile([2, T], f32r, tag="augl")
            nc.scalar.dma_start(augl0_sb[:], augl_d[0])

            # Resident Y^T (fp8), per K-tile-PAIR (DoubleRow contracts two
            # K-tiles per instruction) so q0 starts early, alternated across
            # the two physical HWDGE rings (SP + ACT FIFOs).
            yt_sb = constp.tile([128, NK * SJ], fp8)
            for k in range(NK // 2):
                qeng = nc.sync if k % 2 == 0 else nc.scalar
                qeng.dma_start(yt_sb[:, 2 * k * SJ:(2 * k + 2) * SJ],
                               yt_d[:, 2 * k * SJ:(2 * k + 2) * SJ])
            augr_sb = constp.tile([2, SJ], f32r)
            nc.sync.dma_start(augr_sb[:], augr_d[:])

            # ---- Stage A: cost matrices, one query at a time ----
            for q in range(QC):
                if q == 0:
                    xt_sb, augl_sb = xt0_sb, augl0_sb
                else:
                    # scalar (ACT) HWDGE queue: out of the SP FIFO.
                    xt_sb = xqp.tile([128, NK * T], fp8, tag="xt")
                    nc.scalar.dma_start(xt_sb[:], xt_d[q])
                    augl_sb = augqp.tile([2, T], f32r, tag="augl")
                    nc.scalar.dma_start(augl_sb[:], augl_d[q])

                # [128, NK, *] views for K-tile-pair slicing (DoubleRow).
                xt_k = xt_sb[:].rearrange("p (k t) -> p k t", k=NK)
                yt_k = yt_sb[:].rearrange("p (k sj) -> p k sj", k=NK)
                dq_sb = dqp.tile([128, SJ], f32, tag="dq")
                if q == 0:
                    # k-OUTER while the 8 yt K-pair loads stream in: every
                    # arriving K-pair feeds all 7 chunks (7 PSUM banks live),
                    # so q0's matrix completes with the prologue instead of
                    # ~6us after it. Per-cell accumulation order is unchanged.
                    pss = []
                    for _ci in range(len(CHUNKS)):
                        ps_q0 = psump.tile([128, CH], f32, tag="ps")
                        pss.append(ps_q0)
                    for k in range(NK // 2):
                        for ci, (c0, cw) in enumerate(CHUNKS):
                            nc.tensor.matmul(
                                pss[ci][:, :cw],
                                xt_k[:, 2 * k:2 * k + 2, :],
                                yt_k[:, 2 * k:2 * k + 2, c0:c0 + cw],
                                start=(k == 0),
                                stop=False,
                                perf_mode=DR,
                            )
                    for ci, (c0, cw) in enumerate(CHUNKS):
                        nc.tensor.matmul(
                            pss[ci][:, :cw],
                            augl_sb[:, :],
                            augr_sb[:, c0:c0 + cw],
                            start=False,
                            stop=True,
                        )
                        nc.scalar.copy(dq_sb[:, c0:c0 + cw], pss[ci][:, :cw])
                        nc.sync.dma_start(
                            dsc[q, c0 // T:(c0 + cw) // T]
                            .rearrange("s i j -> i s j"),
                            dq_sb[:, c0:c0 + cw]
                            .rearrange("i (s j) -> i s j", j=T))
                else:
                    for c0, cw in CHUNKS:
                        ps = psump.tile([128, CH], f32, tag="ps")
                        for k in range(NK // 2):
                            # fp8 DoubleRow: two K-tiles contracted per
                            # instruction at 0.5 cycles/row.
                            nc.tensor.matmul(
                                ps[:, :cw],
                                xt_k[:, 2 * k:2 * k + 2, :],
                                yt_k[:, 2 * k:2 * k + 2, c0:c0 + cw],
                                start=(k == 0),
                                stop=False,
                                perf_mode=DR,
                            )
                        # rank-2 fp32 update: + ones*y2[s,j] + x2[i]*ones
                        nc.tensor.matmul(
                            ps[:, :cw],
                            augl_sb[:, :],
                            augr_sb[:, c0:c0 + cw],
                            start=False,
                            stop=True,
                        )
                        nc.scalar.copy(dq_sb[:, c0:c0 + cw], ps[:, :cw])
                        # per-chunk dsc write (chunk widths are whole
                        # s-blocks): the last piece lands ~3.5us after the
                        # last evac instead of a 5us whole-query DMA.
                        nc.sync.dma_start(
                            dsc[q, c0 // T:(c0 + cw) // T]
                            .rearrange("s i j -> i s j"),
                            dq_sb[:, c0:c0 + cw]
                            .rearrange("i (s j) -> i s j", j=T))

            # ---- Stage B: hard-DTW wavefront, 3 batched pair-tiles ----
            out_flat = out_d[:].rearrange("q s -> (q s)")
            for pt, (p0, np_) in enumerate(PT):
                eng = nc.vector  # Pool lacks 2-input TensorTensor on TRN2
                gpool = gbp if pt == 1 else gap
                mudp = mudbp if pt == 1 else mudap
                qa, qb = p0 // S, (p0 + np_ - 1) // S  # query range (aligned)

                r_a = dpp.tile([128, T + 4], f32, tag=f"ra{pt}")
                r_b = dpp.tile([128, T + 4], f32, tag=f"rb{pt}")
                # row 0: [0, BIG, BIG, ...]; r_b border col = BIG.
                # memsets on Pool: keeps them off the DVE critical chain.
                nc.gpsimd.memset(r_a[:np_, 1:T + 1], BIG)
                nc.gpsimd.memset(r_a[:np_, 0:1], 0.0)
                nc.gpsimd.memset(r_b[:np_, 0:1], BIG)

                g_tiles = {}
                for i in range(T):
                    if i % W == 0:
                        g_t = gpool.tile([128, W * T], f32, tag=f"g{pt % 2}")
                        g_tiles[i // W] = g_t
                        # One DMA per window (full SDMA-engine spread).
                        # Pool/SWDGE: idle sequencer, not paced by ACT/SP.
                        # Window 0 of the last stream splits off the final
                        # query so the earlier queries prefetch while q12's
                        # matrix is still being written.
                        if pt == len(PT) - 1 and i == 0:
                            cut = np_ - S
                            nc.gpsimd.dma_start(
                                g_t[:cut, :].rearrange("p (w j) -> p w j", j=T),
                                dsc_p[p0:p0 + cut, i:i + W, :],
                            )
                            nc.gpsimd.dma_start(
                                g_t[cut:np_, :].rearrange(
                                    "p (w j) -> p w j", j=T),
                                dsc_p[p0 + cut:p0 + np_, i:i + W, :],
                            )
                        else:
                            nc.gpsimd.dma_start(
                                g_t[:np_, :].rearrange("p (w j) -> p w j", j=T),
                                dsc_p[p0:p0 + np_, i:i + W, :],
                            )
                    g_t = g_tiles[i // W]
                    prev, cur = (r_a, r_b) if i % 2 == 0 else (r_b, r_a)
                    mud = mudp.tile([128, T], f32, tag=f"m{pt % 2}")
                    eng.tensor_tensor(
                        mud[:np_, :], prev[:np_, 1:T + 1], prev[:np_, 0:T], MIN)
                    eng.tensor_tensor_scan(
                        cur[:np_, 1:T + 1], mud[:np_, :],
                        g_t[:np_, (i % W) * T:(i % W + 1) * T],
                        BIG, MIN, ADD)
                    if i == 0:
                        # row-0 buffer becomes an interior row: border 0 -> BIG
                        eng.memset(prev[:np_, 0:1], BIG)

                final = r_b if T % 2 == 1 else r_a  # T=128 even -> last cur=r_a
                nc.sync.dma_start(out_flat[p0:p0 + np_], final[:np_, T:T + 1])

    ents = getattr(tc, "_perfetto_entries", None)
    if ents:
        _predicted_ns = int(max(e[2] for e in ents))
    nc.compile()
    return nc


def _pack_inputs(X, Yf):
    """Host-side packing into the exact SBUF layouts the kernel DMAs 1:1."""
    f8 = ml_dtypes.float8_e4m3
    # xt[c]: [QC, 128(dk), NK*T] = fp8(-2*X)^T, K-tile-major free dim
    Xp = np.zeros((QPAD, T, DD), np.float32)
    Xp[:Q] = X
    xtq = np.ascontiguousarray(
        (-2.0 * Xp).astype(f8).transpose(0, 2, 1)        # [QPAD, DD, T]
        .reshape(QPAD, NK, 128, T).transpose(0, 2, 1, 3)  # [QPAD, 128, NK, T]
        .reshape(QPAD, 128, NK * T))
    # yt: [128(dk), NK*SJ] = fp8(Y)^T
    yt = np.ascontiguousarray(
        Yf.astype(f8).transpose(2, 0, 1)                 # [DD, S, T]
        .reshape(NK, 128, SJ).transpose(1, 0, 2)         # [128, NK, SJ]
        .reshape(128, NK * SJ))
    # exact fp32 norms
    x2 = np.einsum("qtd,qtd->qt", Xp, Xp, dtype=np.float32)  # [QPAD, T]
    y2 = np.einsum("std,std->st", Yf, Yf, dtype=np.float32)  # [S, T]
    augl = np.zeros((QPAD, 2, T), np.float32)
    augl[:, 0, :] = 1.0
    augl[:, 1, :] = x2
    augr = np.zeros((2, SJ), np.float32)
    augr[0] = y2.reshape(SJ)
    augr[1] = 1.0
    return xtq, yt, augl, augr


def kernel(support_features, support_labels, target_features, n_classes):
    global _built
    from concourse.bass_utils import run_bass_kernel_spmd

    X = np.asarray(target_features, dtype=np.float32)
    Yf = np.asarray(support_features, dtype=np.float32)
    labels = np.asarray(support_labels)
    ncls = int(np.asarray(n_classes))
    assert X.shape == (Q, T, DD) and Yf.shape == (S, T, DD), (
        f"kernel compiled for fixed shapes; got {X.shape}, {Yf.shape}")

    xtq, yt, augl, augr = _pack_inputs(X, Yf)

    if _built is None:
        _built = _build()
    nc = _built

    in_maps = [
        {
            "xt": np.ascontiguousarray(xtq[c * QC:(c + 1) * QC]),
            "yt": yt,
            "augl": np.ascontiguousarray(augl[c * QC:(c + 1) * QC]),
            "augr": augr,
        }
        for c in range(NCORES)
    ]
    res = run_bass_kernel_spmd(nc, in_maps, list(range(NCORES)))
    global _last_result
    _last_result = res
    cum = np.concatenate([res.results[c]["out_cd"] for c in range(NCORES)])[:Q]

    onehot = (labels[:, None] == np.arange(ncls)[None, :]).astype(np.float32)
    counts = np.maximum(onehot.sum(axis=0), 1.0).astype(np.float32)
    logits = -(cum.astype(np.float32) @ onehot) / counts
    return logits.astype(np.float32)



# revision 41
# speedup vs baseline: 1.3246x; 1.3246x over previous
"""Trainium2 Bass kernel: CNN-feature SoftDTW few-shot classifier.

Computes, for Q=100 query sequences and S=25 support sequences (T=128 steps,
D=2048 features): pairwise squared-euclidean cost matrices, soft-DTW alignment
cost per (query, support) pair, then per-class mean distances -> logits.

Key numerical fact: with gamma=0.1 and cost magnitudes ~4096, the reference's
fp32 softmin is bitwise the hard min (exp((m-x)/gamma) underflows for every
non-minimal branch), so the DP is computed with min/add only. Each DP row is
one `tensor_tensor_scan(op0=min, op1=add)` instruction.

Sharding: data-parallel over queries, 13 per core (Q padded 100->104),
supports replicated. Per core:
  - PE: xy = (-2X)@Y^T in bf16 (16 K-tiles) + fp32r rank-2 update adding
    x2[i] + y2[s,j] exactly -> full cost matrix D in PSUM (fp32).
  - ACT: evacuate PSUM -> SBUF; DMA D to DRAM scratch per query.
  - DMA gather: re-layout D from [i, (s,j)] to [(q,s)-partition, i-window, j].
  - DVE: hard-DTW rows: min(up,diag) + scan, 128 rows x 3 pair-streams.
Host: bf16 packing/transposes, x2/y2 sums, final class-mean logits.
"""

import sys

for _p in ("/opt/trn_rl_repo",):
    if _p not in sys.path:
        sys.path.insert(0, _p)

import numpy as np
import ml_dtypes

# Problem shape (hardcoded: harness runs kernel.py standalone)
Q, S, T, DD = 100, 25, 128, 2048
NCORES = 8
QC = 13                 # queries per core; Q padded to 104
QPAD = QC * NCORES
NK = DD // 128          # 16 bf16 contraction tiles
SJ = S * T              # 3200 = flattened (support, j)
B = QC * S              # 325 pairs per core
# DP pair-tile streams aligned to query boundaries (offset, count):
# a stream's first row can run as soon as its LAST query's cost matrix is
# in DRAM, so query-aligned splits start streams as early as possible.
PT = [(0, 75), (75, 125), (200, 125)]   # q0-2 | q3-7 | q8-12
# 3/5/5 split: with the fp8 matmul (~6.7us/query) stream deps land at
# ~29/63/96us; makespan is bounded by dep1 + total DVE scan time, so a
# small first stream starts the DVE chain as early as possible.
W = 16                  # DP row-window per gather DMA
CH = 512                # matmul moving-chunk / PSUM bank width
# all chunks >=256 so the fp32r rank-2 matmul stays at 1 cycle/row
_CW = [512, 512, 512, 512, 512, 384, 256]
CHUNKS = [(sum(_CW[:i]), w) for i, w in enumerate(_CW)]
assert sum(_CW) == SJ
BIG = 1e10

_built = None          # cached compiled Bass program
_last_result = None    # last BassKernelResults (exec_time_ns when traced)
_predicted_ns = None   # Tile cost-model makespan of the per-core program


def _build():
    import concourse.bacc as bacc
    import concourse.mybir as mybir
    import concourse.tile as tile

    f32 = mybir.dt.float32
    f32r = mybir.dt.float32r
    bf16 = mybir.dt.bfloat16
    fp8 = mybir.dt.float8e4
    DR = mybir.MatmulPerfMode.DoubleRow
    MIN = mybir.AluOpType.min
    ADD = mybir.AluOpType.add

    global _predicted_ns
    nc = bacc.Bacc("TRN2", debug=False)

    XW = NK * T
    xt_d = nc.dram_tensor("xt", [QC, 128, XW], fp8, kind="ExternalInput")
    yt_d = nc.dram_tensor("yt", [128, NK * SJ], fp8, kind="ExternalInput")
    augl_d = nc.dram_tensor("augl", [QC, 2, T], f32r, kind="ExternalInput")
    augr_d = nc.dram_tensor("augr", [2, SJ], f32r, kind="ExternalInput")
    out_d = nc.dram_tensor("out_cd", [QC, S], f32, kind="ExternalOutput")
    # cost matrices staged pair-major: [q, s, i, j] -> window reads are
    # single 3-dim APs with 8KB-contiguous runs per pair
    dsc = nc.dram_tensor("dsc", [QC, S, T, T], f32)
    dsc_p = dsc[:].rearrange("q s i j -> (q s) i j")

    with tile.TileContext(nc) as tc:
        with (
            tc.tile_pool(name="const", bufs=1) as constp,
            tc.tile_pool(name="xq", bufs=1) as xqp,
            tc.tile_pool(name="augq", bufs=1) as augqp,
            tc.tile_pool(name="psum", bufs=8, space="PSUM") as psump,
            tc.tile_pool(name="dq", bufs=2) as dqp,
            tc.tile_pool(name="ga", bufs=2) as gap,      # pair-tiles 0 and 2
            tc.tile_pool(name="gb", bufs=2) as gbp,      # pair-tile 1
            tc.tile_pool(name="dp", bufs=1) as dpp,
        ):
            # ACT warm-up: the first ACT op pays ACT_TABLE_LOAD (~1.3us);
            # a dummy copy at t=0 takes it off the q0-evac critical path.
            warm = constp.tile([1, 2], f32)
            nc.gpsimd.memset(warm[:, 0:1], 0.0)
            nc.scalar.copy(warm[:, 1:2], warm[:, 0:1])
            # PE warm-up: the cost model runs the PE at reduced p-state for
            # its first ~3us of activity; burn that ramp on dummy matmuls
            # before q0's operands land instead of on q0 itself.
            wps = psump.tile([128, CH], f32, tag="ps")
            warm2 = constp.tile([16, 16], f32)
            nc.gpsimd.memset(warm2[:], 0.0)
            for _ in range(48):
                nc.tensor.matmul(wps[:16, :16], warm2[:], warm2[:],
                                 start=True, stop=True)

            # q0's xt+augl loads first on the SP ring.
            xt0_sb = xqp.tile([128, XW], fp8, tag="xt0")
            nc.sync.dma_start(xt0_sb[:], xt_d[0])
            augl0_sb = augqp.tile([2, T], f32r, tag="augl0")
            nc.sync.dma_start(augl0_sb[:], augl_d[0])

            # Resident Y^T (fp8), per K-tile-PAIR (DoubleRow contracts two
            # K-tiles per instruction) so q0 starts early, split SP/Pool.
            # NOTHING else rides the ACT SEQ: a dma_start there stalls the
            # PSUM evacs behind its ring-credit wait.
            augr_sb = constp.tile([2, SJ], f32r)
            nc.gpsimd.dma_start(augr_sb[:], augr_d[:])
            yt_sb = constp.tile([128, NK * SJ], fp8)
            for k in range(NK // 2):
                # 3 SP / 3 ACT / 2 Pool: ACT's ring is free until the first
                # evac (~11us) and its SEQ slots retire long before that.
                qeng = (nc.sync, nc.scalar, nc.gpsimd, nc.sync,
                        nc.scalar, nc.gpsimd, nc.sync, nc.scalar)[k]
                qeng.dma_start(yt_sb[:, 2 * k * SJ:(2 * k + 2) * SJ],
                               yt_d[:, 2 * k * SJ:(2 * k + 2) * SJ])

            # ---- Stage A: cost matrices, one query at a time ----
            for q in range(QC):
                if q == 0:
                    xt_sb, augl_sb = xt0_sb, augl0_sb
                else:
                    # Pool SWDGE, all 13 buffered: keeps the xt transfers
                    # off the SP write ring and the ACT evac SEQ; per-q
                    # tags so no slot-wait ever parks at the Pool q head.
                    xt_sb = xqp.tile([128, XW], fp8, tag=f"xt{q}")
                    nc.gpsimd.dma_start(xt_sb[:], xt_d[q])
                    augl_sb = augqp.tile([2, T], f32r, tag=f"augl{q}")
                    nc.gpsimd.dma_start(augl_sb[:], augl_d[q])

                # [128, NK, *] views for K-tile-pair slicing (DoubleRow).
                xt_k = xt_sb[:, :NK * T].rearrange("p (k t) -> p k t", k=NK)
                yt_k = yt_sb[:].rearrange("p (k sj) -> p k sj", k=NK)
                dq_sb = dqp.tile([128, SJ], f32, tag="dq")
                if q == 0:
                    # k-OUTER while the 8 yt K-pair loads stream in: every
                    # arriving K-pair feeds all 7 chunks (7 PSUM banks live),
                    # so q0's matrix completes with the prologue instead of
                    # ~6us after it. Per-cell accumulation order is unchanged.
                    pss = []
                    for _ci in range(len(CHUNKS)):
                        ps_q0 = psump.tile([128, CH], f32, tag="ps")
                        pss.append(ps_q0)
                    for k in range(NK // 2):
                        for ci, (c0, cw) in enumerate(CHUNKS):
                            nc.tensor.matmul(
                                pss[ci][:, :cw],
                                xt_k[:, 2 * k:2 * k + 2, :],
                                yt_k[:, 2 * k:2 * k + 2, c0:c0 + cw],
                                start=(k == 0),
                                stop=False,
                                perf_mode=DR,
                            )
                    for ci, (c0, cw) in enumerate(CHUNKS):
                        nc.tensor.matmul(
                            pss[ci][:, :cw],
                            augl_sb[:, :],
                            augr_sb[:, c0:c0 + cw],
                            start=False,
                            stop=True,
                        )
                        nc.scalar.copy(dq_sb[:, c0:c0 + cw], pss[ci][:, :cw])
                else:
                    for c0, cw in CHUNKS:
                        ps = psump.tile([128, CH], f32, tag="ps")
                        for k in range(NK // 2):
                            # fp8 DoubleRow: two K-tiles contracted per
                            # instruction at 0.5 cycles/row.
                            nc.tensor.matmul(
                                ps[:, :cw],
                                xt_k[:, 2 * k:2 * k + 2, :],
                                yt_k[:, 2 * k:2 * k + 2, c0:c0 + cw],
                                start=(k == 0),
                                stop=False,
                                perf_mode=DR,
                            )
                        # rank-2 fp32 update: + ones*y2[s,j] + x2[i]*ones
                        nc.tensor.matmul(
                            ps[:, :cw],
                            augl_sb[:, :],
                            augr_sb[:, c0:c0 + cw],
                            start=False,
                            stop=True,
                        )
                        nc.scalar.copy(dq_sb[:, c0:c0 + cw], ps[:, :cw])
                if q in (0, 1, 2, 7, 12):
                    # stream-1 critical path: per-chunk writes pipeline the
                    # DRAM staging behind the matmul, so q2's last bytes
                    # land ~1us after its last evac instead of ~6us.
                    for c0, cw in CHUNKS:
                        nc.sync.dma_start(
                            dsc[q, c0 // T:(c0 + cw) // T]
                            .rearrange("s i j -> i s j"),
                            dq_sb[:, c0:c0 + cw]
                            .rearrange("i (s j) -> i s j", j=T))
                else:
                    # ONE whole-query dsc write: each DMA occupies its queue
                    # ~gen+delay+transfer regardless of size, so 7 chunk
                    # writes cost ~14us of queue time vs ~6us for one big.
                    nc.sync.dma_start(
                        dsc[q].rearrange("s i j -> i s j"),
                        dq_sb[:].rearrange("i (s j) -> i s j", j=T))

            # ---- Stage B: hard-DTW wavefront, 3 batched pair-tiles ----
            # One interleaved tensor_tensor_scan per row (2T steps): step
            # (j,0): state = min(diag_j, state) + 0; step (j,1): state =
            # min(up_j, state) + d[i,j].  diag_j = prev[1+2j], up_j =
            # prev[3+2j]; outputs land at cur[2+2j+t] so odd slots hold
            # R[i,j] and the same AP pattern reads them back next row.
            # data1 reads zeros from a Z-track at the head of the g tile:
            # (j,0) -> g[j] = 0, (j,1) -> g[(w+1)*T + j] = d[i,j].
            import bass_rust as _br

            def _dims(ap, dimlist, off_delta):
                c = ap.copy()
                part = list(c.ap)[0]
                c.ap = _br.VecI64Pair(
                    [list(part)] + [list(d) for d in dimlist])
                c.offset = c.offset + off_delta
                return c

            def _scan2(outap, d0, d1):
                nc.vector.add_instruction(
                    mybir.InstTensorScalarPtr(
                        name=nc.vector.bass.get_next_instruction_name(),
                        is_tensor_tensor_scan=True,
                        is_scalar_tensor_tensor=True,
                        op0=MIN, op1=ADD,
                        ins=[nc.vector.lower_ap(d0),
                             nc.vector.lower_ap_or_imm(BIG),
                             nc.vector.lower_ap(d1)],
                        outs=[nc.vector.lower_ap(outap)],
                    ))

            RW = 2 + 2 * T   # row-buffer width
            out_flat = out_d[:].rearrange("q s -> (q s)")
            # stream 0 opens with a small window so its first scan starts
            # right after q2's matrix lands instead of a full 16-row gather
            WLIST = [[4] + [16] * 7 + [12], [16] * 8, [16] * 8]
            rbufs, wininfo = [], []
            for pt, (p0, np_) in enumerate(PT):
                r_i = dpp.tile([128, RW], f32, tag=f"ri{pt}")
                r_a = dpp.tile([128, RW], f32, tag=f"ra{pt}")
                r_b = dpp.tile([128, RW], f32, tag=f"rb{pt}")
                # init row: BIG everywhere, [1] = 0 (the R[-1,-1] corner);
                # interior rows keep [1] = BIG as the left border.
                nc.gpsimd.memset(r_i[:np_, :], BIG)
                nc.gpsimd.memset(r_i[:np_, 1:2], 0.0)
                nc.gpsimd.memset(r_a[:np_, 1:2], BIG)
                nc.gpsimd.memset(r_b[:np_, 1:2], BIG)
                rbufs.append((r_i, r_a, r_b))
                # row -> (window start row, w offset, window len)
                info, base = {}, 0
                for wl in WLIST[pt]:
                    for w in range(wl):
                        info[base + w] = (base, w, wl)
                    base += wl
                wininfo.append(info)

            # Estimated-start-ordered row emission: the DVE engine queue is
            # in-order, so emission priority must match actual readiness --
            # pure round-robin head-of-line-blocks ready stream-1 rows
            # behind stream-2/3 rows whose data hasn't landed.  Order rows
            # by (stream dep time + row index * solo pitch); concurrent
            # streams interleave, which packs out the per-chain write-ack
            # air between a stream's consecutive rows.
            DEP_EST = [34000.0, 72000.0, 104000.0]  # ns, stream data ready
            ROW_PITCH = 420.0
            order = sorted(
                ((DEP_EST[pt] + i * ROW_PITCH, i, pt)
                 for i in range(T) for pt in range(len(PT))))
            g_tiles = [{}, {}, {}]
            for _, i, pt in order:
                if True:
                    p0, np_ = PT[pt]
                    r_i, r_a, r_b = rbufs[pt]
                    w0, w, wl = wininfo[pt][i]
                    if w == 0:
                        gpool = gbp if pt == 1 else gap
                        g_t = gpool.tile([128, T + 16 * T], f32,
                                         tag=f"g{pt % 2}")
                        g_tiles[pt][w0] = g_t
                        # Z-track zeros for the (j,0) scan steps
                        nc.gpsimd.memset(g_t[:np_, 0:T], 0.0)
                        # Gathers: streams 0/2 on the ACT ring (free after
                        # the early xt loads), stream 1 on Pool SWDGE. The
                        # SP ring is left to the cadence-critical dsc
                        # writes; a gather parked at a queue head waiting
                        # for its dsc data would head-of-line-block them.
                        geng = nc.gpsimd
                        geng.dma_start(
                            g_t[:np_, T:T + wl * T]
                            .rearrange("p (w j) -> p w j", j=T),
                            dsc_p[p0:p0 + np_, w0:w0 + wl, :],
                        )
                    g_t = g_tiles[pt][w0]
                    if i == 0:
                        prev, cur = r_i, r_b
                    else:
                        prev, cur = (r_a, r_b) if i % 2 == 0 else (r_b, r_a)
                    _scan2(
                        _dims(cur[:np_], [(2, T), (1, 2)], 2),
                        _dims(prev[:np_], [(2, T), (2, 2)], 1),
                        _dims(g_t[:np_], [(1, T), ((w + 1) * T, 2)], 0),
                    )

            for pt, (p0, np_) in enumerate(PT):
                final = rbufs[pt][1]  # T=128 even -> last cur=r_a
                nc.sync.dma_start(
                    out_flat[p0:p0 + np_], final[:np_, RW - 1:RW])

    ents = getattr(tc, "_perfetto_entries", None)
    if ents:
        _predicted_ns = int(max(e[2] for e in ents))
    nc.compile()
    return nc


def _pack_inputs(X, Yf):
    """Host-side packing into the exact SBUF layouts the kernel DMAs 1:1."""
    f8 = ml_dtypes.float8_e4m3
    # xt[c]: [QC, 128(dk), NK*T] = fp8(-2*X)^T, K-tile-major free dim
    Xp = np.zeros((QPAD, T, DD), np.float32)
    Xp[:Q] = X
    xtq = np.ascontiguousarray(
        (-2.0 * Xp).astype(f8).transpose(0, 2, 1)        # [QPAD, DD, T]
        .reshape(QPAD, NK, 128, T).transpose(0, 2, 1, 3)  # [QPAD, 128, NK, T]
        .reshape(QPAD, 128, NK * T))
    # exact fp32 norms
    x2 = np.einsum("qtd,qtd->qt", Xp, Xp, dtype=np.float32)  # [QPAD, T]
    y2 = np.einsum("std,std->st", Yf, Yf, dtype=np.float32)  # [S, T]
    augl = np.zeros((QPAD, 2, T), np.float32)
    augl[:, 0, :] = 1.0
    augl[:, 1, :] = x2
    # yt: [128(dk), NK*SJ] = fp8(Y)^T
    yt = np.ascontiguousarray(
        Yf.astype(f8).transpose(2, 0, 1)                 # [DD, S, T]
        .reshape(NK, 128, SJ).transpose(1, 0, 2)         # [128, NK, SJ]
        .reshape(128, NK * SJ))
    augr = np.zeros((2, SJ), np.float32)
    augr[0] = y2.reshape(SJ)
    augr[1] = 1.0
    return xtq, yt, augl, augr


def kernel(support_features, support_labels, target_features, n_classes):
    global _built
    from concourse.bass_utils import run_bass_kernel_spmd

    X = np.asarray(target_features, dtype=np.float32)
    Yf = np.asarray(support_features, dtype=np.float32)
    labels = np.asarray(support_labels)
    ncls = int(np.asarray(n_classes))
    assert X.shape == (Q, T, DD) and Yf.shape == (S, T, DD), (
        f"kernel compiled for fixed shapes; got {X.shape}, {Yf.shape}")

    xtq, yt, augl, augr = _pack_inputs(X, Yf)

    if _built is None:
        _built = _build()
    nc = _built

    in_maps = [
        {
            "xt": np.ascontiguousarray(xtq[c * QC:(c + 1) * QC]),
            "yt": yt,
            "augl": np.ascontiguousarray(augl[c * QC:(c + 1) * QC]),
            "augr": augr,
        }
        for c in range(NCORES)
    ]
    res = run_bass_kernel_spmd(nc, in_maps, list(range(NCORES)))
    global _last_result
    _last_result = res
    cum = np.concatenate([res.results[c]["out_cd"] for c in range(NCORES)])[:Q]

    onehot = (labels[:, None] == np.arange(ncls)[None, :]).astype(np.float32)
    counts = np.maximum(onehot.sum(axis=0), 1.0).astype(np.float32)
    logits = -(cum.astype(np.float32) @ onehot) / counts
    return logits.astype(np.float32)

